# revision 20
# baseline (speedup 1.0000x reference)
"""MultiHeadSimilarity kernel for 8 Trainium2 NeuronCores.

Reference computation (per batch b):
    Q = wq @ x[b];  K = wk @ y[b]                       (channel-mixing matmuls)
    per head h (d=64):  A = relu(Qh^T Kh) * scale, masked by xy_mask
    C = A @ Kh^T, normalized per-row by 1/max(sum(mask, y), 1)
    out = wo @ (0.5 * (Q + C))

Sharding: data-parallel over batch; 16 batches / 8 cores = 2 per core.
Weights replicated. No cross-core communication.

Device algorithm (fp16 compute, fp32 PSUM):
  - Q = wqT.T @ x, K = wkT.T @ y (natural-layout fp16 matmuls).
  - KT (y on partitions, needed as the C-matmul stationary) comes from a
    DMA XBAR transpose of K — zero tensor-engine cycles (the baseline
    recomputed it as a second projection).
  - n_el row counts and inv = 1/(8*max(n,1)) are computed on the HOST
    (the mask is host-visible); inv is DMA'd and partition-broadcast.
  - A is computed transposed (y on partitions) per head pair; two heads
    pack the 128-wide PE array as 64-row groups (quadrant ping-pong hides
    weight loads). relu+mask fuses into one op, statically balanced over
    THREE engines: DVE fused scalar_tensor_tensor, or ACT relu + DVE
    multiply, or ACT relu + GPSIMD multiply. The mask is fp8 (halves its
    DMA; exact 0/1 values).
  - C accumulates two heads per PSUM bank (64-col groups); normalization
    and the E = 0.5(Q+C) merge ride DVE; 0.5 is folded into woT on host.
"""
import sys

if "/opt/trn_rl_repo" not in sys.path:
    sys.path.insert(0, "/opt/trn_rl_repo")

import numpy as np

import concourse.tile as tile
from concourse import bacc, mybir
from concourse.bass_utils import run_bass_kernel_spmd

F16 = mybir.dt.float16
F32 = mybir.dt.float32
F8 = mybir.dt.float8e4
AL = mybir.AluOpType
RELU = mybir.ActivationFunctionType.Relu
DR = mybir.MatmulPerfMode.DoubleRow

N_CORES = 8
B, U, LX, LY, H, D = 16, 512, 1024, 1024, 8, 64
BPC = B // N_CORES          # batches per core
KB = U // 128               # 4  k-tiles over channels
HP = H // 2                 # 4  head pairs
YT = LY // 128              # 8  y tiles
XH = LX // 512              # 2  x halves
INV_SCALE = float(D) ** 0.5  # 8.0; attention scale 1/8 folded into host inv

TRACE = False
LDW_OPT = False
DR_GROUPS = 0
_CACHE = {}

# measured per-op costs (ns) for the static 3-engine balancer
C_DVE_STT = 1223.0   # fused relu+mask fd1024, PSUM f32 in
C_ACT_RELU = 1114.0  # ACT relu fd1024 PSUM -> SBUF
C_DVE_TT2 = 665.0    # DVE f16 mask mult fd1024, 2x
C_GP_TT = 2030.0     # GPSIMD mask mult fd1024
C_ACT_CP1024 = 1137.0
C_ACT_CP512 = 710.0
C_DVE_CP1024 = 1200.0
C_DVE_CP512 = 670.0


class Bal3:
    """Static greedy balancer across DVE / ACT / GP."""

    def __init__(self, nc):
        self.nc = nc
        self.t = {"v": 0.0, "s": 0.0, "g": 0.0}

    def add(self, eng, ns):
        self.t[eng] += ns

    def copy(self, dst, src, fd):
        dve = C_DVE_CP1024 if fd >= 1024 else C_DVE_CP512
        act = C_ACT_CP1024 if fd >= 1024 else C_ACT_CP512
        if self.t["v"] + dve <= self.t["s"] + act:
            self.t["v"] += dve
            self.nc.vector.tensor_copy(dst, src)
        else:
            self.t["s"] += act
            self.nc.scalar.copy(dst, src)

    def relu_mask_pair(self, out, a_ps, mtf_b, tmp_pool, name, force_stt=False):
        """out[(128,2,512)] = relu(a_ps) * mtf_b (two heads, one y-tile).

        option v:  fused DVE scalar_tensor_tensor (PSUM in; only option that
                   can write f8 without a rate penalty)
        option sd: ACT relu -> f16 + DVE f16 multiply (2x)
        """
        v_end = self.t["v"] + C_DVE_STT
        sd_end = max(self.t["s"] + C_ACT_RELU, self.t["v"] + C_DVE_TT2)
        if force_stt or v_end <= sd_end:
            self.t["v"] += C_DVE_STT
            self.nc.vector.scalar_tensor_tensor(out[:], a_ps[:], 0.0, mtf_b,
                                                AL.max, AL.mult)
            return
        ra = tmp_pool.tile([128, 2, 512], F16, tag="ra", name=name)
        self.nc.scalar.activation(ra[:], a_ps[:], RELU)
        self.t["s"] += C_ACT_RELU
        self.t["v"] += C_DVE_TT2
        self.nc.vector.tensor_tensor(out[:], ra[:], mtf_b, AL.mult)


def _patch_ldw_opt():
    """Flip walrus --enable-ldw-opt so back-to-back matmuls sharing a
    stationary operand don't reload the PE array each time."""
    from concourse import bass_utils as bu
    if getattr(bu.bir_verify_and_optimise, "_ldw_patched", False):
        return
    orig = bu.bir_verify_and_optimise

    def patched(*args, **kwargs):
        import subprocess
        orig_run = subprocess.run

        def run_hook(cmd, *a, **kw):
            if isinstance(cmd, list):
                cmd = ["--enable-ldw-opt=true" if c == "--enable-ldw-opt=false"
                       else c for c in cmd]
            return orig_run(cmd, *a, **kw)

        subprocess.run = run_hook
        try:
            return orig(*args, **kwargs)
        finally:
            subprocess.run = orig_run

    patched._ldw_patched = True
    bu.bir_verify_and_optimise = patched


def _build():
    if LDW_OPT:
        _patch_ldw_opt()
    nc = bacc.Bacc("TRN2", target_bir_lowering=False, debug=False,
                   num_devices=N_CORES)
    x_e = nc.dram_tensor("x", [BPC, U, LX], F16, kind="ExternalInput")
    y_e = nc.dram_tensor("y", [BPC, U, LY], F16, kind="ExternalInput")
    mt_e = nc.dram_tensor("mt", [BPC, LY, LX], F16, kind="ExternalInput")
    inv_e = nc.dram_tensor("inv", [BPC, LX], F32, kind="ExternalInput")
    w_all_e = nc.dram_tensor("w_all", [3, U, U], F16, kind="ExternalInput")
    o_e = nc.dram_tensor("o", [BPC, U, LX], F32, kind="ExternalOutput")

    with tile.TileContext(nc) as tc:
        _emit(nc, tc, x_e, y_e, mt_e, inv_e, w_all_e, o_e)
    nc.compile()
    return nc


def _emit(nc, tc, x_e, y_e, mt_e, inv_e, w_all_e, o_e):
    import contextlib
    bal = Bal3(nc)
    ctx = contextlib.ExitStack()
    with ctx:
        wp = ctx.enter_context(tc.tile_pool(name="wp", bufs=1))
        io = ctx.enter_context(tc.tile_pool(name="io", bufs=2))
        pr = ctx.enter_context(tc.tile_pool(name="pr", bufs=2))
        sm = ctx.enter_context(tc.tile_pool(name="sm", bufs=2))
        amp = ctx.enter_context(tc.tile_pool(name="amp", bufs=4))
        osp = ctx.enter_context(tc.tile_pool(name="osp", bufs=3))
        pa = ctx.enter_context(tc.tile_pool(name="pa", bufs=3, space="PSUM"))
        pc = ctx.enter_context(tc.tile_pool(name="pc", bufs=2, space="PSUM"))

        # weights, loaded once
        WQT = wp.tile([128, KB, U], F16, tag="wqt")
        WKT = wp.tile([128, KB, U], F16, tag="wkt")
        WOT = wp.tile([128, KB, U], F16, tag="wot")
        for wi, w_t in enumerate((WQT, WKT, WOT)):
            nc.scalar.dma_start(
                w_t[:], w_all_e.ap()[wi].rearrange("(k p) o -> p k o", p=128))

        for b in range(BPC):
            # ---- input loads ----
            X = io.tile([128, KB, LX], F16, tag="x", name=f"x{b}")
            Y = io.tile([128, KB, LY], F16, tag="y", name=f"y{b}")
            for k in range(KB):
                nc.sync.dma_start(X[:, k, :], x_e.ap()[b, k * 128:(k + 1) * 128, :])
            for k in range(KB):
                nc.gpsimd.dma_start(Y[:, k, :], y_e.ap()[b, k * 128:(k + 1) * 128, :])
            MTF8 = io.tile([128, YT, LX], F16, tag="mtf8", name=f"mtf8{b}")
            for t in range(YT):
                (nc.sync if t % 2 == 0 else nc.gpsimd).dma_start(
                    MTF8[:, t, :], mt_e.ap()[b, t * 128:(t + 1) * 128, :])
            invr = sm.tile([1, LX], F32, tag="invr", name=f"invr{b}")
            nc.sync.dma_start(invr[:], inv_e.ap()[b:b + 1, :])
            invb = sm.tile([128, LX], F32, tag="invb", name=f"invb{b}")
            nc.gpsimd.partition_broadcast(invb[:], invr[:])

            # ---- projections: Q = wqT.T @ x, K = wkT.T @ y ----
            Q = pr.tile([128, KB, LX], F16, tag="q", name=f"q{b}")
            K = pr.tile([128, KB, LY], F16, tag="k", name=f"k{b}")
            for m in range(KB):
                ps = pa.tile([128, 2, 512], F32, tag="a", name=f"pjq{b}_{m}")
                for k in range(KB):
                    for n in range(XH):
                        nc.tensor.matmul(
                            ps[:, n, :], WQT[:, k, m * 128:(m + 1) * 128],
                            X[:, k, n * 512:(n + 1) * 512],
                            start=(k == 0), stop=(k == KB - 1))
                bal.copy(Q[:, m, :], ps[:], 1024)
            for m in range(KB):
                ps = pa.tile([128, 2, 512], F32, tag="a", name=f"pjk{b}_{m}")
                for k in range(KB):
                    for n in range(XH):
                        nc.tensor.matmul(
                            ps[:, n, :], WKT[:, k, m * 128:(m + 1) * 128],
                            Y[:, k, n * 512:(n + 1) * 512],
                            start=(k == 0), stop=(k == KB - 1))
                bal.copy(K[:, m, :], ps[:], 1024)
            # KT = y.T @ wkT : two l-tiles share one psum pair slot
            KT = pr.tile([128, YT, U], F16, tag="kt", name=f"kt{b}")
            for lt2 in range(YT // 2):
                ps = pa.tile([128, 2, 512], F32, tag="a", name=f"pkt{b}_{lt2}")
                for i in range(2):
                    lt = lt2 * 2 + i
                    for k in range(KB):
                        nc.tensor.matmul(ps[:, i, :],
                                         Y[:, k, lt * 128:(lt + 1) * 128],
                                         WKT[:, k, :512],
                                         start=(k == 0), stop=(k == KB - 1))
                bal.copy(KT[:, lt2 * 2:lt2 * 2 + 2, :], ps[:], 1024)

            # ---- attention ----
            # groups 0..4 run fp16 C (pairs split ACT+DVE); groups 5..7 use
            # fp8 DoubleRow C (pairs fused f8 STT on DVE, halves C cycles)
            E = pr.tile([128, KB, LX], F16, tag="e", name=f"e{b}")
            gi = 0
            for hp in range(HP):
                for xh in range(XH):
                    xs = slice(xh * 512, (xh + 1) * 512)
                    use_dr = gi >= 8 - DR_GROUPS
                    gi += 1
                    if not use_dr:
                        # both heads accumulate into ONE bank: j0 at cols 0-63,
                        # j1 at 64-127
                        C = pc.tile([128, 512], F32, tag="c",
                                    name=f"c_{b}_{hp}_{xh}")
                        for yt in range(YT):
                            A = pa.tile([128, 2, 512], F32, tag="a",
                                        name=f"a_{b}_{hp}_{xh}_{yt}")
                            for j in range(2):
                                hs = slice(64 * j, 64 * (j + 1))
                                nc.tensor.matmul(
                                    A[:, j, :], K[hs, hp, yt * 128:(yt + 1) * 128],
                                    Q[hs, hp, xs], start=True, stop=True)
                            Am = amp.tile([128, 2, 512], F16, tag="am", bufs=6,
                                          name=f"am_{b}_{hp}_{xh}_{yt}")
                            mtf_b = MTF8[:, yt, xs].unsqueeze(1).broadcast_to(
                                (128, 2, 512))
                            bal.relu_mask_pair(Am, A, mtf_b, amp,
                                               f"ra_{b}_{hp}_{xh}_{yt}")
                            for j in range(2):
                                hs = slice(64 * j, 64 * (j + 1))
                                nc.tensor.matmul(
                                    C[hs, :],
                                    KT[:, yt, hp * 128 + 64 * j: hp * 128 + 64 * (j + 1)],
                                    Am[:, j, :], start=(yt == 0),
                                    stop=(yt == YT - 1), skip_group_check=True)
                        Et = amp.tile([128, 512], F16, tag="et",
                                      name=f"et_{b}_{hp}_{xh}")
                        nc.vector.tensor_tensor(Et[:], C[:], invb[:, xs], AL.mult)
                        bal.add("v", C_DVE_CP512)
                        nc.vector.tensor_tensor(E[:, hp, xs], Et[:],
                                                Q[:, hp, xs], AL.add)
                        bal.add("v", 400.0)
                    else:
                        # fp8 DR: per-head C psum tiles, M=128, half garbage
                        C0 = pc.tile([128, 512], F32, tag="c",
                                     name=f"c0_{b}_{hp}_{xh}")
                        C1 = pc.tile([128, 512], F32, tag="c",
                                     name=f"c1_{b}_{hp}_{xh}")
                        for t in range(YT // 2):
                            Am8 = amp.tile([128, 2, 2, 512], F8, tag="am8",
                                           bufs=4, name=f"am8_{b}_{hp}_{xh}_{t}")
                            for i in range(2):
                                yt = 2 * t + i
                                A = pa.tile([128, 2, 512], F32, tag="a",
                                            name=f"a_{b}_{hp}_{xh}_{yt}")
                                for j in range(2):
                                    hs = slice(64 * j, 64 * (j + 1))
                                    nc.tensor.matmul(
                                        A[:, j, :],
                                        K[hs, hp, yt * 128:(yt + 1) * 128],
                                        Q[hs, hp, xs], start=True, stop=True)
                                mtf_b = MTF8[:, yt, xs].unsqueeze(1).broadcast_to(
                                    (128, 2, 512))
                                bal.relu_mask_pair(Am8[:, i, :, :], A, mtf_b,
                                                   amp, f"ra_{b}_{hp}_{xh}_{yt}",
                                                   force_stt=True)
                            ktp = KT8[:, 2 * t:2 * t + 2,
                                      hp * 128:(hp + 1) * 128]
                            for j, Cps in ((0, C0), (1, C1)):
                                nc.tensor.matmul(Cps[:], ktp, Am8[:, :, j, :],
                                                 start=(t == 0),
                                                 stop=(t == YT // 2 - 1),
                                                 perf_mode=DR)
                        for j, Cps in ((0, C0), (1, C1)):
                            hs = slice(64 * j, 64 * (j + 1))
                            Et = amp.tile([128, 512], F16, tag="et",
                                          name=f"et_{b}_{hp}_{xh}_{j}")
                            nc.vector.tensor_tensor(Et[hs, :], Cps[hs, :],
                                                    invb[hs, xs], AL.mult)
                            bal.add("v", C_DVE_CP512)
                            nc.vector.tensor_tensor(E[hs, hp, xs], Et[hs, :],
                                                    Q[hs, hp, xs], AL.add)
                            bal.add("v", 400.0)

            # ---- output projection ----
            for m in range(KB):
                ps = pa.tile([128, 2, 512], F32, tag="a", name=f"po{b}_{m}")
                for k in range(KB):
                    for n in range(XH):
                        nc.tensor.matmul(ps[:, n, :],
                                         WOT[:, k, m * 128:(m + 1) * 128],
                                         E[:, k, n * 512:(n + 1) * 512],
                                         start=(k == 0), stop=(k == KB - 1))
                oS = osp.tile([128, LX], F32, tag="os", name=f"os{b}_{m}")
                for n in range(XH):
                    bal.copy(oS[:, n * 512:(n + 1) * 512], ps[:, n, :], 512)
                    nc.sync.dma_start(
                        o_e.ap()[b, m * 128:(m + 1) * 128, n * 512:(n + 1) * 512],
                        oS[:, n * 512:(n + 1) * 512])


def _get_nc():
    if "nc" not in _CACHE:
        _CACHE["nc"] = _build()
    return _CACHE["nc"]


def kernel(x, y, xy_mask, wq, wk, wo):
    import ml_dtypes
    nc = _get_nc()
    xf = x.astype(np.float16)
    yf = y.astype(np.float16)
    mtt = np.ascontiguousarray(
        xy_mask.transpose(0, 2, 1)).astype(np.float16)
    nel = np.maximum(xy_mask.sum(axis=2), 1).astype(np.float32)  # (B, LX)
    inv = (1.0 / (INV_SCALE * nel)).astype(np.float32)
    w_all = np.stack([wq.T, wk.T, (0.5 * wo).T]).astype(np.float16)
    w_all = np.ascontiguousarray(w_all)
    in_maps = [
        {"x": xf[c * BPC:(c + 1) * BPC], "y": yf[c * BPC:(c + 1) * BPC],
         "mt": mtt[c * BPC:(c + 1) * BPC], "inv": inv[c * BPC:(c + 1) * BPC],
         "w_all": w_all}
        for c in range(N_CORES)
    ]
    res = run_bass_kernel_spmd(nc, in_maps, list(range(N_CORES)), trace=TRACE)
    if TRACE:
        _CACHE["last_exec_time_ns"] = res.exec_time_ns
        _CACHE["last_profile_json"] = res.profile_json
    return np.concatenate([res.results[c]["o"] for c in range(N_CORES)], axis=0)


# revision 22
# speedup vs baseline: 1.0059x; 1.0059x over previous
"""MultiHeadSimilarity kernel for 8 Trainium2 NeuronCores.

Reference computation (per batch b):
    Q = wq @ x[b];  K = wk @ y[b]                       (channel-mixing matmuls)
    per head h (d=64):  A = relu(Qh^T Kh) * scale, masked by xy_mask
    C = A @ Kh^T, normalized per-row by 1/max(sum(mask, y), 1)
    out = wo @ (0.5 * (Q + C))

Sharding: data-parallel over batch; 16 batches / 8 cores = 2 per core.
Weights replicated. No cross-core communication.

Device algorithm (fp16 compute, fp32 PSUM accumulation):
  - Q = wqT.T @ x, K = wkT.T @ y, KT = y.T @ wkT (natural-layout matmuls;
    the K transpose needed by the C-contraction is computed as a second
    projection - cheaper in practice than DMA XBAR transpose, and the
    extra dense matmuls keep the PE's HAM clock-gate warm).
  - n_el row counts and inv = 1/(8*max(n_el,1)) are computed on the HOST
    (the mask is host-visible; saves the ones^T @ mask matmuls and the
    on-chip reciprocal chain); inv is DMA'd per batch and
    partition-broadcast on GPSIMD.
  - A is computed transposed (y on partitions) per head; relu+mask fuse
    into one DVE scalar_tensor_tensor, or split ACT relu + DVE multiply,
    statically balanced between the two engines. Two heads pack the
    128-wide PE array (K=64 row groups for the A matmuls / M=64 col
    groups for the C matmuls) so weight loads overlap streaming.
  - C accumulates two heads per PSUM bank (64-col groups); 0.5 is folded
    into woT on the host.

Measured notes (this hardware): fp8 DoubleRow matmuls trigger a ~50% PE
utilization clamp (HAM activity_1) with tens-of-us hysteresis, erasing
their 2x contraction win everywhere - all-fp16 tensor work is faster.
DMA XBAR transpose of K costs more than recomputing KT as a projection.
"""
import sys

if "/opt/trn_rl_repo" not in sys.path:
    sys.path.insert(0, "/opt/trn_rl_repo")

import numpy as np

import concourse.tile as tile
from concourse import bacc, mybir
from concourse.bass_utils import run_bass_kernel_spmd

F16 = mybir.dt.float16
F32 = mybir.dt.float32
F8 = mybir.dt.float8e4
AL = mybir.AluOpType
RELU = mybir.ActivationFunctionType.Relu
DR = mybir.MatmulPerfMode.DoubleRow

N_CORES = 8
B, U, LX, LY, H, D = 16, 512, 1024, 1024, 8, 64
BPC = B // N_CORES          # batches per core
KB = U // 128               # 4  k-tiles over channels
HP = H // 2                 # 4  head pairs
YT = LY // 128              # 8  y tiles
XH = LX // 512              # 2  x halves
INV_SCALE = float(D) ** 0.5  # 8.0; attention scale 1/8 folded into host inv

TRACE = False
LDW_OPT = False
_CACHE = {}

# measured per-op costs (ns) for the static 3-engine balancer
C_DVE_STT = 1223.0   # fused relu+mask fd1024, PSUM f32 in
C_ACT_RELU = 1114.0  # ACT relu fd1024 PSUM -> SBUF
C_DVE_TT2 = 665.0    # DVE f16 mask mult fd1024, 2x
C_GP_TT = 2030.0     # GPSIMD mask mult fd1024
C_ACT_CP1024 = 1137.0
C_ACT_CP512 = 710.0
C_DVE_CP1024 = 1200.0
C_DVE_CP512 = 670.0


class Bal3:
    """Static greedy balancer across DVE / ACT / GP."""

    def __init__(self, nc):
        self.nc = nc
        self.t = {"v": 0.0, "s": 0.0, "g": 0.0}

    def add(self, eng, ns):
        self.t[eng] += ns

    def copy(self, dst, src, fd):
        dve = C_DVE_CP1024 if fd >= 1024 else C_DVE_CP512
        act = C_ACT_CP1024 if fd >= 1024 else C_ACT_CP512
        if self.t["v"] + dve <= self.t["s"] + act:
            self.t["v"] += dve
            self.nc.vector.tensor_copy(dst, src)
        else:
            self.t["s"] += act
            self.nc.scalar.copy(dst, src)

    def relu_mask_pair(self, out, a_ps, mtf_b, tmp_pool, name, force_stt=False):
        """out[(128,2,512)] = relu(a_ps) * mtf_b (two heads, one y-tile).

        option v:  fused DVE scalar_tensor_tensor (PSUM in; only option that
                   can write f8 without a rate penalty)
        option sd: ACT relu -> f16 + DVE f16 multiply (2x)
        """
        v_end = self.t["v"] + C_DVE_STT
        sd_end = max(self.t["s"] + C_ACT_RELU, self.t["v"] + C_DVE_TT2)
        if force_stt or v_end <= sd_end:
            self.t["v"] += C_DVE_STT
            self.nc.vector.scalar_tensor_tensor(out[:], a_ps[:], 0.0, mtf_b,
                                                AL.max, AL.mult)
            return
        ra = tmp_pool.tile([128, 2, 512], F16, tag="ra", name=name)
        self.nc.scalar.activation(ra[:], a_ps[:], RELU)
        self.t["s"] += C_ACT_RELU
        self.t["v"] += C_DVE_TT2
        self.nc.vector.tensor_tensor(out[:], ra[:], mtf_b, AL.mult)


def _patch_ldw_opt():
    """Flip walrus --enable-ldw-opt so back-to-back matmuls sharing a
    stationary operand don't reload the PE array each time."""
    from concourse import bass_utils as bu
    if getattr(bu.bir_verify_and_optimise, "_ldw_patched", False):
        return
    orig = bu.bir_verify_and_optimise

    def patched(*args, **kwargs):
        import subprocess
        orig_run = subprocess.run

        def run_hook(cmd, *a, **kw):
            if isinstance(cmd, list):
                cmd = ["--enable-ldw-opt=true" if c == "--enable-ldw-opt=false"
                       else c for c in cmd]
            return orig_run(cmd, *a, **kw)

        subprocess.run = run_hook
        try:
            return orig(*args, **kwargs)
        finally:
            subprocess.run = orig_run

    patched._ldw_patched = True
    bu.bir_verify_and_optimise = patched


def _build():
    if LDW_OPT:
        _patch_ldw_opt()
    nc = bacc.Bacc("TRN2", target_bir_lowering=False, debug=False,
                   num_devices=N_CORES)
    x_e = nc.dram_tensor("x", [BPC, U, LX], F16, kind="ExternalInput")
    y_e = nc.dram_tensor("y", [BPC, U, LY], F16, kind="ExternalInput")
    mt_e = nc.dram_tensor("mt", [BPC, LY, LX], F16, kind="ExternalInput")
    inv_e = nc.dram_tensor("inv", [BPC, LX], F32, kind="ExternalInput")
    w_all_e = nc.dram_tensor("w_all", [3, U, U], F16, kind="ExternalInput")
    o_e = nc.dram_tensor("o", [BPC, U, LX], F32, kind="ExternalOutput")

    with tile.TileContext(nc) as tc:
        _emit(nc, tc, x_e, y_e, mt_e, inv_e, w_all_e, o_e)
    nc.compile()
    return nc


def _emit(nc, tc, x_e, y_e, mt_e, inv_e, w_all_e, o_e):
    import contextlib
    bal = Bal3(nc)
    ctx = contextlib.ExitStack()
    with ctx:
        wp = ctx.enter_context(tc.tile_pool(name="wp", bufs=1))
        io = ctx.enter_context(tc.tile_pool(name="io", bufs=2))
        pr = ctx.enter_context(tc.tile_pool(name="pr", bufs=2))
        sm = ctx.enter_context(tc.tile_pool(name="sm", bufs=2))
        amp = ctx.enter_context(tc.tile_pool(name="amp", bufs=4))
        osp = ctx.enter_context(tc.tile_pool(name="osp", bufs=3))
        pa = ctx.enter_context(tc.tile_pool(name="pa", bufs=3, space="PSUM"))
        pc = ctx.enter_context(tc.tile_pool(name="pc", bufs=2, space="PSUM"))

        # weights, loaded once
        WQT = wp.tile([128, KB, U], F16, tag="wqt")
        WKT = wp.tile([128, KB, U], F16, tag="wkt")
        WOT = wp.tile([128, KB, U], F16, tag="wot")
        for wi, w_t in enumerate((WQT, WKT, WOT)):
            nc.scalar.dma_start(
                w_t[:], w_all_e.ap()[wi].rearrange("(k p) o -> p k o", p=128))

        for b in range(BPC):
            # ---- input loads ----
            X = io.tile([128, KB, LX], F16, tag="x", name=f"x{b}")
            Y = io.tile([128, KB, LY], F16, tag="y", name=f"y{b}")
            for k in range(KB):
                nc.sync.dma_start(X[:, k, :], x_e.ap()[b, k * 128:(k + 1) * 128, :])
            for k in range(KB):
                nc.gpsimd.dma_start(Y[:, k, :], y_e.ap()[b, k * 128:(k + 1) * 128, :])
            MTF8 = io.tile([128, YT, LX], F16, tag="mtf8", name=f"mtf8{b}")
            for t in range(YT):
                (nc.sync if t % 2 == 0 else nc.gpsimd).dma_start(
                    MTF8[:, t, :], mt_e.ap()[b, t * 128:(t + 1) * 128, :])
            invr = sm.tile([1, LX], F32, tag="invr", name=f"invr{b}")
            nc.sync.dma_start(invr[:], inv_e.ap()[b:b + 1, :])
            invb = sm.tile([128, LX], F32, tag="invb", name=f"invb{b}")
            nc.gpsimd.partition_broadcast(invb[:], invr[:])

            # ---- projections: Q = wqT.T @ x, K = wkT.T @ y ----
            Q = pr.tile([128, KB, LX], F16, tag="q", name=f"q{b}")
            K = pr.tile([128, KB, LY], F16, tag="k", name=f"k{b}")
            for m in range(KB):
                ps = pa.tile([128, 2, 512], F32, tag="a", name=f"pjq{b}_{m}")
                for k in range(KB):
                    for n in range(XH):
                        nc.tensor.matmul(
                            ps[:, n, :], WQT[:, k, m * 128:(m + 1) * 128],
                            X[:, k, n * 512:(n + 1) * 512],
                            start=(k == 0), stop=(k == KB - 1))
                bal.copy(Q[:, m, :], ps[:], 1024)
            for m in range(KB):
                ps = pa.tile([128, 2, 512], F32, tag="a", name=f"pjk{b}_{m}")
                for k in range(KB):
                    for n in range(XH):
                        nc.tensor.matmul(
                            ps[:, n, :], WKT[:, k, m * 128:(m + 1) * 128],
                            Y[:, k, n * 512:(n + 1) * 512],
                            start=(k == 0), stop=(k == KB - 1))
                bal.copy(K[:, m, :], ps[:], 1024)
            # KT = y.T @ wkT : two l-tiles share one psum pair slot
            KT = pr.tile([128, YT, U], F16, tag="kt", name=f"kt{b}")
            for lt2 in range(YT // 2):
                ps = pa.tile([128, 2, 512], F32, tag="a", name=f"pkt{b}_{lt2}")
                for i in range(2):
                    lt = lt2 * 2 + i
                    for k in range(KB):
                        nc.tensor.matmul(ps[:, i, :],
                                         Y[:, k, lt * 128:(lt + 1) * 128],
                                         WKT[:, k, :512],
                                         start=(k == 0), stop=(k == KB - 1))
                bal.copy(KT[:, lt2 * 2:lt2 * 2 + 2, :], ps[:], 1024)

            # ---- attention ----
            E = pr.tile([128, KB, LX], F16, tag="e", name=f"e{b}")
            for hp in range(HP):
                for xh in range(XH):
                    xs = slice(xh * 512, (xh + 1) * 512)
                    # both heads accumulate into ONE bank: j0 at cols 0-63,
                    # j1 at 64-127
                    C = pc.tile([128, 512], F32, tag="c",
                                name=f"c_{b}_{hp}_{xh}")
                    for yt in range(YT):
                        A = pa.tile([128, 2, 512], F32, tag="a",
                                    name=f"a_{b}_{hp}_{xh}_{yt}")
                        for j in range(2):
                            hs = slice(64 * j, 64 * (j + 1))
                            nc.tensor.matmul(
                                A[:, j, :], K[hs, hp, yt * 128:(yt + 1) * 128],
                                Q[hs, hp, xs], start=True, stop=True)
                        Am = amp.tile([128, 2, 512], F16, tag="am", bufs=6,
                                      name=f"am_{b}_{hp}_{xh}_{yt}")
                        mtf_b = MTF8[:, yt, xs].unsqueeze(1).broadcast_to(
                            (128, 2, 512))
                        bal.relu_mask_pair(Am, A, mtf_b, amp,
                                           f"ra_{b}_{hp}_{xh}_{yt}")
                        for j in range(2):
                            hs = slice(64 * j, 64 * (j + 1))
                            nc.tensor.matmul(
                                C[hs, :],
                                KT[:, yt, hp * 128 + 64 * j: hp * 128 + 64 * (j + 1)],
                                Am[:, j, :], start=(yt == 0),
                                stop=(yt == YT - 1), skip_group_check=True)
                    Et = amp.tile([128, 512], F16, tag="et",
                                  name=f"et_{b}_{hp}_{xh}")
                    nc.vector.tensor_tensor(Et[:], C[:], invb[:, xs], AL.mult)
                    bal.add("v", C_DVE_CP512)
                    nc.vector.tensor_tensor(E[:, hp, xs], Et[:],
                                            Q[:, hp, xs], AL.add)
                    bal.add("v", 400.0)

            # ---- output projection ----
            for m in range(KB):
                ps = pa.tile([128, 2, 512], F32, tag="a", name=f"po{b}_{m}")
                for k in range(KB):
                    for n in range(XH):
                        nc.tensor.matmul(ps[:, n, :],
                                         WOT[:, k, m * 128:(m + 1) * 128],
                                         E[:, k, n * 512:(n + 1) * 512],
                                         start=(k == 0), stop=(k == KB - 1))
                oS = osp.tile([128, LX], F32, tag="os", name=f"os{b}_{m}")
                for n in range(XH):
                    bal.copy(oS[:, n * 512:(n + 1) * 512], ps[:, n, :], 512)
                    nc.sync.dma_start(
                        o_e.ap()[b, m * 128:(m + 1) * 128, n * 512:(n + 1) * 512],
                        oS[:, n * 512:(n + 1) * 512])


def _get_nc():
    if "nc" not in _CACHE:
        _CACHE["nc"] = _build()
    return _CACHE["nc"]


def kernel(x, y, xy_mask, wq, wk, wo):
    import ml_dtypes
    nc = _get_nc()
    xf = x.astype(np.float16)
    yf = y.astype(np.float16)
    mtt = np.ascontiguousarray(
        xy_mask.transpose(0, 2, 1)).astype(np.float16)
    nel = np.maximum(xy_mask.sum(axis=2), 1).astype(np.float32)  # (B, LX)
    inv = (1.0 / (INV_SCALE * nel)).astype(np.float32)
    w_all = np.stack([wq.T, wk.T, (0.5 * wo).T]).astype(np.float16)
    w_all = np.ascontiguousarray(w_all)
    in_maps = [
        {"x": xf[c * BPC:(c + 1) * BPC], "y": yf[c * BPC:(c + 1) * BPC],
         "mt": mtt[c * BPC:(c + 1) * BPC], "inv": inv[c * BPC:(c + 1) * BPC],
         "w_all": w_all}
        for c in range(N_CORES)
    ]
    res = run_bass_kernel_spmd(nc, in_maps, list(range(N_CORES)), trace=TRACE)
    if TRACE:
        _CACHE["last_exec_time_ns"] = res.exec_time_ns
        _CACHE["last_profile_json"] = res.profile_json
    return np.concatenate([res.results[c]["o"] for c in range(N_CORES)], axis=0)
